# revision 10
# baseline (speedup 1.0000x reference)
"""Trainium2 Bass kernel for 4-layer GAT (nn_GAT_4layer).

Strategy (8 NeuronCores, SPMD):
  - Nodes are relabeled/padded to 20480 = 8 cores x 2560, balancing in-degree
    across 160 dst-tiles of 128 nodes (greedy bin packing). Edges (+self loops)
    are sharded by destination block, sorted by dst, padded per dst-tile to a
    common chunk count (CPT chunks of 128 edge slots).
  - Per layer each core keeps a replicated node table tab[N,320] with rows
    [h(256) | es(4) | ed(4) | pad]; per dst-tile one dma_gather pulls the
    tile's own 128 rows (for ed[dst]) + all edge source rows.
  - Attention: e = leaky_relu(es[src]+ed[dst]); ed[dst] expanded per edge with
    a one-hot matmul (MT), exp on ACT (no segment max needed: the U/den ratio
    is shift-invariant), messages scaled per head, and scatter-added into
    PSUM with a one-hot matmul (M). den is carried as 4 extra columns.
  - BN: batch stats via per-core partial sums + tiny AllGather; affine+relu
    fused into one ACT op per 128-feature chunk (feature-major after a PE
    transpose). Next layer's table rows are produced by a node-sharded matmul
    and replicated with a 26MB AllGather.
  - Final MLP head is node-sharded and feature-major; host unpermutes.
"""

import numpy as np

NCORES = 8
NPC = 2560                     # nodes per core
NPAD = NCORES * NPC            # 20480
TPC = 20                       # dst tiles per core
TILE = 128
H, D = 4, 64
FEAT = 256                     # H*D
TROW = 320                     # table row f32 elements (1280B, multiple of 256B)
NEG_SLOPE = 0.2
BN_EPS = 1e-5
NGENE = 978
NGPAD = 1024


# --------------------------------------------------------------------------
# host-side graph preparation
# --------------------------------------------------------------------------

def _build_partition(edge_index: np.ndarray, n_nodes: int):
    import heapq

    src0 = edge_index[0].astype(np.int64)
    dst0 = edge_index[1].astype(np.int64)
    deg = np.bincount(dst0, minlength=n_nodes) + 1

    n_tiles = NCORES * TPC
    pads_per_core = (NPAD - n_nodes) // NCORES
    last_tile_valid = TILE - pads_per_core

    order = np.argsort(-deg, kind="stable")
    tile_load = np.zeros(n_tiles, dtype=np.int64)
    tile_count = np.zeros(n_tiles, dtype=np.int64)
    tile_cap = np.full(n_tiles, TILE, dtype=np.int64)
    tile_cap[TPC - 1 :: TPC] = last_tile_valid
    members: list[list[int]] = [[] for _ in range(n_tiles)]
    heap = [(0, t) for t in range(n_tiles)]
    heapq.heapify(heap)
    for nid in order:
        popped = []
        while True:
            load, t = heapq.heappop(heap)
            if tile_count[t] < tile_cap[t]:
                break
            popped.append((load, t))
        for item in popped:
            heapq.heappush(heap, item)
        members[t].append(int(nid))
        tile_count[t] += 1
        tile_load[t] += int(deg[nid])
        heapq.heappush(heap, (int(tile_load[t]), t))

    new2old = np.full(NPAD, -1, dtype=np.int64)
    for g in range(n_tiles):
        base = g * TILE
        for s, nid in enumerate(members[g]):
            new2old[base + s] = nid
    old2new = np.full(n_nodes, -1, dtype=np.int64)
    valid = new2old >= 0
    old2new[new2old[valid]] = np.nonzero(valid)[0]
    assert (old2new >= 0).all()

    src_n = old2new[src0]
    dst_n = old2new[dst0]
    loop = np.arange(NPAD, dtype=np.int64)
    src_all = np.concatenate([src_n, loop])
    dst_all = np.concatenate([dst_n, loop])

    tile_of = dst_all // TILE
    order_e = np.argsort(tile_of * (NPAD + 1) + dst_all, kind="stable")
    src_s = src_all[order_e]
    dst_s = dst_all[order_e]
    counts = np.bincount(tile_of, minlength=n_tiles)
    cpt = int(np.ceil(counts.max() / TILE))
    slots = cpt * TILE

    gsrc = np.zeros((n_tiles, slots), dtype=np.int64)
    gdstrel = np.full((n_tiles, slots), -1, dtype=np.int64)
    offs = np.concatenate([[0], np.cumsum(counts)])
    for g in range(n_tiles):
        a, b = offs[g], offs[g + 1]
        k = b - a
        gsrc[g, :k] = src_s[a:b]
        gdstrel[g, :k] = dst_s[a:b] - g * TILE
    return dict(
        new2old=new2old, old2new=old2new, cpt=cpt, gsrc=gsrc, gdstrel=gdstrel,
        last_tile_valid=last_tile_valid,
    )


def _fold_weights(params):
    out = []
    for l in range(4):
        W = np.asarray(params[f"W{l}"], dtype=np.float32)
        asrc = np.asarray(params[f"a_src{l}"], dtype=np.float32)
        adst = np.asarray(params[f"a_dst{l}"], dtype=np.float32)
        As = np.zeros((FEAT, H), dtype=np.float32)
        Ad = np.zeros((FEAT, H), dtype=np.float32)
        for h in range(H):
            As[h * D : (h + 1) * D, h] = asrc[h]
            Ad[h * D : (h + 1) * D, h] = adst[h]
        out.append(np.concatenate([W, W @ As, W @ Ad], axis=1).astype(np.float32))
    return out  # list of [fin, 264]


GMAX = 1024  # max idxs per dma_gather instruction (HW ring limit)


def _wrap_idx(iv: np.ndarray) -> np.ndarray:
    """int16 idx vector -> [128, len/16]; wrapped (i%16, i//16) per GMAX
    segment, replicated x8 across partition groups."""
    segs = []
    for a in range(0, iv.shape[0], GMAX):
        seg = iv[a : a + GMAX]
        w = seg.reshape(seg.shape[0] // 16, 16).T.astype(np.int16)
        segs.append(w)
    w = np.concatenate(segs, axis=1)
    return np.tile(w, (8, 1))


# --------------------------------------------------------------------------
# bass program
# --------------------------------------------------------------------------

def _build_bass(cpt: int, last_tile_valid: int, n_valid: int):
    import concourse.bacc as bacc
    import concourse.bass as bass
    import concourse.tile as tile
    from concourse import mybir
    from concourse.library_config import mlp
    from contextlib import ExitStack

    F32 = mybir.dt.float32
    U8 = mybir.dt.uint8
    I16 = mybir.dt.int16
    AF = mybir.ActivationFunctionType
    OP = mybir.AluOpType
    X = mybir.AxisListType.X

    NIDX = (cpt + 1) * TILE          # gather idx count per tile (own rows + edges)
    GMAX = 1024
    IDXC = NIDX // 16                # idx cols per tile in wrapped layout
    MCOL = cpt * TILE                # M/MT cols per tile
    RG = [[list(range(NCORES))]][0]  # replica groups [[0..7]]

    nc = bacc.Bacc("TRN2")

    # ---- DRAM tensors ----
    xT = nc.dram_tensor("xT", [TILE, NPAD], F32, kind="ExternalInput")
    w0 = nc.dram_tensor("w0", [128, 264], F32, kind="ExternalInput")
    wk = [None] + [
        nc.dram_tensor(f"w{l}", [128, 2 * 264], F32, kind="ExternalInput")
        for l in range(1, 4)
    ]  # layers 1-3: [128, k0(264)|k1(264)]
    hw1 = nc.dram_tensor("hw1", [128, 512], F32, kind="ExternalInput")   # (k,m) packed
    hw2 = nc.dram_tensor("hw2", [128, 256], F32, kind="ExternalInput")   # k packed
    hw3 = nc.dram_tensor("hw3", [128, NGPAD], F32, kind="ExternalInput")
    b1c = nc.dram_tensor("b1c", [128, 2], F32, kind="ExternalInput")
    b2c = nc.dram_tensor("b2c", [128, 1], F32, kind="ExternalInput")
    b3c = nc.dram_tensor("b3c", [128, 8], F32, kind="ExternalInput")
    bnrow = nc.dram_tensor("bnrow", [1, 2048], F32, kind="ExternalInput")
    ident = nc.dram_tensor("ident", [128, 128], F32, kind="ExternalInput")
    ones8 = nc.dram_tensor("ones8", [8, 1], F32, kind="ExternalInput")
    gidx = nc.dram_tensor("gidx", [TPC, 128, IDXC], I16, kind="ExternalInput")
    m8d = nc.dram_tensor("m8d", [TPC, 128, MCOL], U8, kind="ExternalInput")
    mt8d = nc.dram_tensor("mt8d", [TPC, 128, MCOL], U8, kind="ExternalInput")

    tabA = nc.dram_tensor("tabA", [NPAD, TROW], F32, kind="Internal", addr_space="Shared")
    tabB = nc.dram_tensor("tabB", [NPAD, TROW], F32, kind="Internal", addr_space="Shared")
    stage = nc.dram_tensor("stage", [NPC, TROW], F32, kind="Internal")
    stats_in = nc.dram_tensor("stats_in", [1, 512], F32, kind="Internal")
    stats_all = nc.dram_tensor("stats_all", [8, 512], F32, kind="Internal", addr_space="Shared")
    gbb = nc.dram_tensor("gbb", [1, 512], F32, kind="Internal")
    outT = nc.dram_tensor("outT", [NGPAD, NPC], F32, kind="ExternalOutput")

    with tile.TileContext(nc) as tc, ExitStack() as ctx:
        const = ctx.enter_context(tc.tile_pool(name="const", bufs=1))
        persist = ctx.enter_context(tc.tile_pool(name="persist", bufs=1))
        edgep = ctx.enter_context(tc.tile_pool(name="edgep", bufs=2))
        mcast = ctx.enter_context(tc.tile_pool(name="mcast", bufs=2))
        chunkp = ctx.enter_context(tc.tile_pool(name="chunkp", bufs=4))
        p2 = ctx.enter_context(tc.tile_pool(name="p2", bufs=3))
        statp = ctx.enter_context(tc.tile_pool(name="statp", bufs=2))
        psU = ctx.enter_context(tc.tile_pool(name="psU", bufs=2, space="PSUM"))
        psE = ctx.enter_context(tc.tile_pool(name="psE", bufs=2, space="PSUM"))
        psT = ctx.enter_context(tc.tile_pool(name="psT", bufs=2, space="PSUM"))
        psH = ctx.enter_context(tc.tile_pool(name="psH", bufs=2, space="PSUM"))

        nc.gpsimd.load_library(mlp)

        # ---- constants ----
        identT = const.tile([128, 128], F32)
        nc.sync.dma_start(identT[:], ident[:])
        ones8T = const.tile([8, 1], F32)
        nc.sync.dma_start(ones8T[:], ones8[:])
        bnrowT = const.tile([1, 2048], F32)
        nc.sync.dma_start(bnrowT[:], bnrow[:])
        w0T = const.tile([128, 264], F32)
        nc.sync.dma_start(w0T[:], w0[:])
        wkT = [None]
        for l in range(1, 4):
            t_ = const.tile([128, 2 * 264], F32, tag=f"wk{l}")
            nc.sync.dma_start(t_[:], wk[l][:])
            wkT.append(t_)
        hw1T = const.tile([128, 512], F32)
        nc.sync.dma_start(hw1T[:], hw1[:])
        hw2T = const.tile([128, 256], F32)
        nc.sync.dma_start(hw2T[:], hw2[:])
        hw3T = const.tile([128, NGPAD], F32)
        nc.sync.dma_start(hw3T[:], hw3[:])
        b1cT = const.tile([128, 2], F32)
        nc.sync.dma_start(b1cT[:], b1c[:])
        b2cT = const.tile([128, 1], F32)
        nc.sync.dma_start(b2cT[:], b2c[:])
        b3cT = const.tile([128, 8], F32)
        nc.sync.dma_start(b3cT[:], b3c[:])
        gidxT = const.tile([128, TPC * IDXC], I16)
        nc.sync.dma_start(
            gidxT[:].rearrange("p (t c) -> p t c", t=TPC),
            gidx[:].rearrange("t p c -> p t c"),
        )

        zT = [persist.tile([128, NPC], F32, tag=f"z{c}", name=f"z{c}") for c in range(2)]
        aggT = [persist.tile([128, NPC], F32, tag=f"agg{c}", name=f"agg{c}") for c in range(2)]
        sxT = persist.tile([128, 2 * TPC], F32, tag="sx")
        sqT = persist.tile([128, 2 * TPC], F32, tag="sq")

        # zero the stage pad columns once (AG moves them; keep deterministic)
        epsT = const.tile([1, 1], F32)
        nc.vector.memset(epsT[:], BN_EPS)
        zpad = const.tile([128, TROW - 264], F32)
        nc.vector.memset(zpad[:], 0.0)
        for t in range(TPC):
            nc.sync.dma_start(stage[t * TILE : (t + 1) * TILE, 264:TROW], zpad[:])

        # ---- L0: replicated full table build: tabA = x @ W'0 ----
        for g in range(NPAD // TILE):
            xt = p2.tile([128, 128], F32, tag="xt")
            nc.sync.dma_start(xt[:], xT[:, g * TILE : (g + 1) * TILE])
            hp = psH.tile([128, 264], F32, space="PSUM", tag="hps")
            nc.tensor.matmul(hp[:], xt[:], w0T[:], start=True, stop=True)
            hs = p2.tile([128, 264], F32, tag="hs")
            nc.vector.tensor_copy(hs[:], hp[:])
            nc.sync.dma_start(tabA[g * TILE : (g + 1) * TILE, 0:264], hs[:])
            nc.sync.dma_start(tabA[g * TILE : (g + 1) * TILE, 264:TROW], zpad[:])

        # ---- layers ----
        for l in range(4):
            tab = tabA if l % 2 == 0 else tabB
            tab_next = tabB if l % 2 == 0 else tabA

            for t in range(TPC):
                gsl = edgep.tile([128, cpt + 1, TROW], F32, tag="G")
                for s0 in range(0, NIDX, GMAX):
                    sn = min(GMAX, NIDX - s0)
                    nc.gpsimd.dma_gather(
                        gsl[:, s0 // TILE : (s0 + sn) // TILE, :], tab[:],
                        gidxT[:, t * IDXC + s0 // 16 : t * IDXC + (s0 + sn) // 16],
                        sn, sn, TROW,
                    )
                m8t = edgep.tile([128, MCOL], U8, tag="m8")
                nc.sync.dma_start(m8t[:], m8d[t, :, :])
                mt8t = edgep.tile([128, MCOL], U8, tag="mt8")
                nc.sync.dma_start(mt8t[:], mt8d[t, :, :])

                Up = psU.tile([128, 260], F32, space="PSUM", tag="Up")
                mf = mtf = None
                for k in range(cpt):
                    if k % 4 == 0:
                        w_ = min(4, cpt - k) * TILE
                        mf = mcast.tile([128, 512], F32, tag="mf")
                        nc.vector.tensor_copy(mf[:, 0:w_], m8t[:, k * TILE : k * TILE + w_])
                        mtf = mcast.tile([128, 512], F32, tag="mtf")
                        nc.vector.tensor_copy(mtf[:, 0:w_], mt8t[:, k * TILE : k * TILE + w_])
                    kk = (k % 4) * TILE
                    edp = psE.tile([128, 4], F32, space="PSUM", tag="edp")
                    nc.tensor.matmul(
                        edp[:], mtf[:, kk : kk + TILE], gsl[:, 0, 260:264],
                        start=True, stop=True,
                    )
                    V = chunkp.tile([128, 260], F32, tag="V")
                    e_ = chunkp.tile([128, 4], F32, tag="e")
                    nc.vector.tensor_tensor(
                        out=e_[:], in0=gsl[:, k + 1, 256:260], in1=edp[:], op=OP.add
                    )
                    lt = chunkp.tile([128, 4], F32, tag="lt")
                    nc.vector.tensor_scalar(
                        out=lt[:], in0=e_[:], scalar1=NEG_SLOPE, scalar2=None, op0=OP.mult
                    )
                    lr = chunkp.tile([128, 4], F32, tag="lr")
                    nc.vector.tensor_tensor(out=lr[:], in0=lt[:], in1=e_[:], op=OP.max)
                    nc.scalar.activation(V[:, 256:260], lr[:], AF.Exp)
                    for h in range(H):
                        src = gsl[:, k + 1, h * D : (h + 1) * D]
                        dstv = V[:, h * D : (h + 1) * D]
                        sc = V[:, 256 + h : 257 + h]
                        if h < 2:
                            nc.scalar.activation(dstv, src, AF.Copy, scale=sc)
                        else:
                            nc.vector.tensor_scalar(
                                out=dstv, in0=src, scalar1=sc, scalar2=None, op0=OP.mult
                            )
                    nc.tensor.matmul(
                        Up[:], mf[:, kk : kk + TILE], V[:],
                        start=(k == 0), stop=(k == cpt - 1),
                    )

                # tile epilogue: agg = U/den, transpose, stats
                rden = chunkp.tile([128, 4], F32, tag="rden")
                nc.vector.reciprocal(rden[:], Up[:, 256:260])
                aggs = p2.tile([128, 256], F32, tag="aggs")
                for h in range(H):
                    nc.vector.tensor_scalar(
                        out=aggs[:, h * D : (h + 1) * D],
                        in0=Up[:, h * D : (h + 1) * D],
                        scalar1=rden[:, h : h + 1], scalar2=None, op0=OP.mult,
                    )
                lim = last_tile_valid if t == TPC - 1 else TILE
                for c in range(2):
                    trp = psT.tile([128, 128], F32, space="PSUM", tag="trp")
                    nc.tensor.transpose(trp[:], aggs[:, c * 128 : (c + 1) * 128], identT[:])
                    nc.vector.tensor_copy(aggT[c][:, t * TILE : (t + 1) * TILE], trp[:])
                    col = c * TPC + t
                    nc.vector.tensor_reduce(
                        out=sxT[:, col : col + 1],
                        in_=aggT[c][:, t * TILE : t * TILE + lim],
                        axis=X, op=OP.add,
                    )
                    sqs = chunkp.tile([128, 128], F32, tag="sqs")
                    nc.scalar.activation(
                        sqs[:, 0:lim], aggT[c][:, t * TILE : t * TILE + lim],
                        AF.Square, accum_out=sqT[:, col : col + 1],
                    )

            # ---- stats exchange + bn coefficients ----
            stats4 = statp.tile([128, 4], F32, tag="stats4")
            nc.vector.tensor_reduce(out=stats4[:, 0:1], in_=sxT[:, 0:TPC], axis=X, op=OP.add)
            nc.vector.tensor_reduce(out=stats4[:, 1:2], in_=sxT[:, TPC : 2 * TPC], axis=X, op=OP.add)
            nc.vector.tensor_reduce(out=stats4[:, 2:3], in_=sqT[:, 0:TPC], axis=X, op=OP.add)
            nc.vector.tensor_reduce(out=stats4[:, 3:4], in_=sqT[:, TPC : 2 * TPC], axis=X, op=OP.add)
            nc.sync.dma_start(stats_in[0, :].rearrange("(j f) -> f j", f=128), stats4[:])
            nc.gpsimd.collective_compute(
                "AllGather", OP.bypass, replica_groups=RG,
                ins=[stats_in[:]], outs=[stats_all[:]],
            )
            s8 = statp.tile([8, 512], F32, tag="s8")
            nc.sync.dma_start(s8[:], stats_all[:])
            sps = psH.tile([1, 512], F32, space="PSUM", tag="hps")
            nc.tensor.matmul(sps[:], ones8T[:], s8[:], start=True, stop=True)
            srow = statp.tile([1, 512], F32, tag="srow")
            nc.vector.tensor_copy(srow[:], sps[:])
            mu = statp.tile([1, 256], F32, tag="mu")
            nc.vector.tensor_scalar(out=mu[:], in0=srow[:, 0:256], scalar1=1.0 / n_valid, scalar2=None, op0=OP.mult)
            ex2 = statp.tile([1, 256], F32, tag="ex2")
            nc.vector.tensor_scalar(out=ex2[:], in0=srow[:, 256:512], scalar1=1.0 / n_valid, scalar2=None, op0=OP.mult)
            var_ = statp.tile([1, 256], F32, tag="var")
            nc.vector.tensor_tensor(out=var_[:], in0=mu[:], in1=mu[:], op=OP.mult)
            nc.vector.tensor_tensor(out=var_[:], in0=ex2[:], in1=var_[:], op=OP.subtract)
            std = statp.tile([1, 256], F32, tag="std")
            nc.scalar.activation(std[:], var_[:], AF.Sqrt, bias=epsT[:])
            rstd = statp.tile([1, 256], F32, tag="rstd")
            nc.vector.reciprocal(rstd[:], std[:])
            gbrow = statp.tile([1, 512], F32, tag="gbrow")
            nc.vector.tensor_tensor(out=gbrow[:, 0:256], in0=bnrowT[:, l * 512 : l * 512 + 256], in1=rstd[:], op=OP.mult)
            tmp = statp.tile([1, 256], F32, tag="tmp")
            nc.vector.tensor_tensor(out=tmp[:], in0=mu[:], in1=gbrow[:, 0:256], op=OP.mult)
            nc.vector.tensor_tensor(out=gbrow[:, 256:512], in0=bnrowT[:, l * 512 + 256 : l * 512 + 512], in1=tmp[:], op=OP.subtract)
            nc.sync.dma_start(gbb[:], gbrow[:])
            gbc = statp.tile([128, 4], F32, tag="gbc")
            nc.sync.dma_start(gbc[:], gbb[0, :].rearrange("(j f) -> f j", f=128))

            # ---- pass 2: bn+relu (feature-major), z accum, next table ----
            for t in range(TPC):
                relu_c = []
                for c in range(2):
                    rt = p2.tile([128, 128], F32, tag=f"reluT{c}")
                    nc.scalar.activation(
                        rt[:], aggT[c][:, t * TILE : (t + 1) * TILE], AF.Relu,
                        scale=gbc[:, c : c + 1], bias=gbc[:, 2 + c : 3 + c],
                    )
                    relu_c.append(rt)
                    if l == 0:
                        nc.vector.tensor_copy(zT[c][:, t * TILE : (t + 1) * TILE], rt[:])
                    else:
                        nc.vector.tensor_add(
                            out=zT[c][:, t * TILE : (t + 1) * TILE],
                            in0=zT[c][:, t * TILE : (t + 1) * TILE], in1=rt[:],
                        )
                if l < 3:
                    hp = psH.tile([128, 264], F32, space="PSUM", tag="hps")
                    nc.tensor.matmul(hp[:], relu_c[0][:], wkT[l + 1][:, 0:264], start=True, stop=False)
                    nc.tensor.matmul(hp[:], relu_c[1][:], wkT[l + 1][:, 264:528], start=False, stop=True)
                    hs = p2.tile([128, 264], F32, tag="hs")
                    nc.vector.tensor_copy(hs[:], hp[:])
                    nc.sync.dma_start(stage[t * TILE : (t + 1) * TILE, 0:264], hs[:])

            if l < 3:
                nc.gpsimd.collective_compute(
                    "AllGather", OP.bypass, replica_groups=RG,
                    ins=[stage[:]], outs=[tab_next[:]],
                )

        # ---- head: feature-major, node-sharded ----
        GS = min(512, NPC)
        for n in range(NPC // GS):
            nsl = slice(n * GS, (n + 1) * GS)
            o1 = []
            for m in range(2):
                pp = psH.tile([128, GS], F32, space="PSUM", tag="hps")
                nc.tensor.matmul(pp[:], hw1T[:, (0 * 2 + m) * 128 : (0 * 2 + m + 1) * 128], zT[0][:, nsl], start=True, stop=False)
                nc.tensor.matmul(pp[:], hw1T[:, (1 * 2 + m) * 128 : (1 * 2 + m + 1) * 128], zT[1][:, nsl], start=False, stop=True)
                o1m = p2.tile([128, GS], F32, tag=f"o1{m}")
                nc.scalar.activation(o1m[:], pp[:], AF.Relu, bias=b1cT[:, m : m + 1])
                o1.append(o1m)
            pp2 = psH.tile([128, GS], F32, space="PSUM", tag="hps")
            nc.tensor.matmul(pp2[:], hw2T[:, 0:128], o1[0][:], start=True, stop=False)
            nc.tensor.matmul(pp2[:], hw2T[:, 128:256], o1[1][:], start=False, stop=True)
            o2 = p2.tile([128, GS], F32, tag="o2")
            nc.scalar.activation(o2[:], pp2[:], AF.Relu, bias=b2cT[:, 0:1])
            for m in range(8):
                pp3 = psH.tile([128, GS], F32, space="PSUM", tag="hps")
                nc.tensor.matmul(pp3[:], hw3T[:, m * 128 : (m + 1) * 128], o2[:], start=True, stop=True)
                o3 = p2.tile([128, GS], F32, tag="o3")
                nc.scalar.activation(o3[:], pp3[:], AF.Identity, bias=b3cT[:, m : m + 1])
                nc.sync.dma_start(outT[m * 128 : (m + 1) * 128, nsl], o3[:])

    nc.compile()
    return nc


# --------------------------------------------------------------------------
# entry point
# --------------------------------------------------------------------------

def _prepare(x, ei, params):
    n_nodes = x.shape[0]

    part = _build_partition(ei, n_nodes)
    cpt = part["cpt"]
    ltv = part["last_tile_valid"]
    new2old = part["new2old"]
    valid = new2old >= 0

    # permuted padded transposed input
    xp = np.zeros((NPAD, x.shape[1]), dtype=np.float32)
    xp[valid] = x[new2old[valid]]
    xT_v = np.ascontiguousarray(xp.T)  # [128, 20480]

    Wf = _fold_weights(params)
    w_in = {"w0": Wf[0]}
    for l in range(1, 4):
        w_in[f"w{l}"] = np.ascontiguousarray(
            np.concatenate([Wf[l][0:128, :], Wf[l][128:256, :]], axis=1)
        )  # [128, 528]

    W1 = np.asarray(params["lin1_w"], dtype=np.float32)
    W2 = np.asarray(params["lin2_w"], dtype=np.float32)
    W3 = np.asarray(params["lin3_w"], dtype=np.float32)
    hw1_v = np.zeros((128, 512), dtype=np.float32)
    for k in range(2):
        for m in range(2):
            hw1_v[:, (k * 2 + m) * 128 : (k * 2 + m + 1) * 128] = W1[k * 128 : (k + 1) * 128, m * 128 : (m + 1) * 128]
    hw2_v = np.zeros((128, 256), dtype=np.float32)
    for k in range(2):
        hw2_v[:, k * 128 : (k + 1) * 128] = W2[k * 128 : (k + 1) * 128, :]
    hw3_v = np.zeros((128, NGPAD), dtype=np.float32)
    hw3_v[:, :NGENE] = W3
    b1c_v = np.ascontiguousarray(np.asarray(params["lin1_b"], np.float32).reshape(2, 128).T)
    b2c_v = np.ascontiguousarray(np.asarray(params["lin2_b"], np.float32).reshape(1, 128).T)
    b3_p = np.zeros(NGPAD, dtype=np.float32)
    b3_p[:NGENE] = np.asarray(params["lin3_b"], np.float32)
    b3c_v = np.ascontiguousarray(b3_p.reshape(8, 128).T)
    bnrow_v = np.concatenate(
        [
            np.concatenate([np.asarray(params[f"bn_g{l}"], np.float32), np.asarray(params[f"bn_b{l}"], np.float32)])
            for l in range(4)
        ]
    ).reshape(1, 2048)
    ident_v = np.eye(128, dtype=np.float32)
    ones8_v = np.ones((8, 1), dtype=np.float32)

    # per-core edge constants
    NIDX = (cpt + 1) * TILE
    gidx_cores, m8_cores, mt8_cores = [], [], []
    ar128 = np.arange(128)
    for c in range(NCORES):
        gx = np.zeros((TPC, 128, NIDX // 16), dtype=np.int16)
        m8 = np.zeros((TPC, 128, cpt * TILE), dtype=np.uint8)
        mt8 = np.zeros((TPC, 128, cpt * TILE), dtype=np.uint8)
        for t in range(TPC):
            g = c * TPC + t
            iv = np.concatenate(
                [g * TILE + ar128, part["gsrc"][g]]
            ).astype(np.int16)
            gx[t] = _wrap_idx(iv)
            dr = part["gdstrel"][g].reshape(cpt, TILE)  # [cpt, 128e]
            oh = (dr[:, :, None] == ar128[None, None, :])  # [cpt, e, n]
            m8[t] = oh.transpose(1, 0, 2).reshape(TILE, cpt * TILE)
            mt8[t] = oh.transpose(2, 0, 1).reshape(TILE, cpt * TILE)
        gidx_cores.append(gx)
        m8_cores.append(m8)
        mt8_cores.append(mt8)

    nc = _build_bass(cpt, ltv, n_nodes)

    common = dict(
        xT=xT_v, hw1=hw1_v, hw2=hw2_v, hw3=hw3_v, b1c=b1c_v, b2c=b2c_v,
        b3c=b3c_v, bnrow=bnrow_v, ident=ident_v, ones8=ones8_v, **w_in,
    )
    in_maps = [
        dict(common, gidx=gidx_cores[c], m8d=m8_cores[c], mt8d=mt8_cores[c])
        for c in range(NCORES)
    ]

    return nc, in_maps, part


def _finish(core_outs, part):
    out_pad = np.zeros((NPAD, NGENE), dtype=np.float32)
    for c in range(NCORES):
        out_pad[c * NPC : (c + 1) * NPC] = np.asarray(core_outs[c])[:NGENE, :].T
    return out_pad[part["old2new"]].astype(np.float32)


def kernel(**inputs) -> np.ndarray:
    x = np.asarray(inputs["x"], dtype=np.float32)
    ei = np.asarray(inputs["edge_index"]).astype(np.int64)
    params = {k: np.asarray(v) for k, v in inputs["params"].items()}
    nc, in_maps, part = _prepare(x, ei, params)

    from concourse.bass_utils import run_bass_kernel_spmd

    res = run_bass_kernel_spmd(nc, in_maps, core_ids=list(range(NCORES)))
    return _finish([r["outT"] for r in res.results], part)
